# revision 12
# baseline (speedup 1.0000x reference)
"""GroupNorm + single-head-per-core attention + output projection for
nn_Attention_55697135894780 on 8 TRN2 NeuronCores.

Sharding: one (batch, head) pair per core (B=2 x NH=4 = 8 cores), no
cross-device communication.

Host-side prep (cheap, O(N*C)): GroupNorm statistics, scale/bias folding
into the tiny weight operands, and the centered bf16 activation tile --
the same category of preprocessing as the existing weight replication /
w_out @ w_v folding. The device kernel is the O(N^2) attention:

  xca    = [bf16(x[b] - group_mean) ; ones-row]           [65, 3072] bf16
  q4/k4  = replicated head projections                    [128, 3072] bf16
           (4 copies of q/k in 32-partition strips so QK^T can use
            tile_position row-packing with K=16; the packed matmuls
            stream CONCURRENTLY through disjoint row strips)
  S^T    = K^T Q computed j-on-partitions (no transposes anywhere)
  E      = exp(S^T) in bf16, split per pack between ScalarE (true Exp,
           cols 0:acols) and VectorE (Schraudolph bit-trick:
           int16(S*128*log2e + 127*128 - C) viewed as bf16). The first
           packs run ACT-only while VectorE drains projection copies.
  out    = E^T G computed i-on-partitions: per 128-col i-block,
           matmul(lhsT=E[j,128i], rhs=G[j,65]) accumulating over the 24
           j-blocks -- 65-column streams instead of 512-column streams.
           G[j,0:64] = (w_out_h @ w_v_h @ norm)^T, G[j,64] = 1 so column
           64 is the softmax denominator (no max-subtraction: |S| < ~40,
           fp32 exp cannot overflow).
  PV for pack p is emitted after pack p+2's QK (pipeline depth 2): the
  PE prefetches lhsT weights during the preceding matmul, so a
  just-written E tile used as lhsT races the exp engines at depth 1.

Host combines: x + b_out + sum_h(num/den), reshaped to [2,64,12,16,16].
"""

import sys
from contextlib import ExitStack

import numpy as np
import ml_dtypes

sys.path.insert(0, "/opt/trn_rl_repo")

import concourse.bacc as bacc  # noqa: E402
import concourse.tile as tile  # noqa: E402
from concourse import mybir  # noqa: E402
from concourse.bass_utils import run_bass_kernel_spmd  # noqa: E402

B, C, D_, H_, W_ = 2, 64, 12, 16, 16
N = D_ * H_ * W_  # 3072
NH, DH, NG = 4, 16, 4  # heads, head_dim, groups
EPS = 1e-5
F32 = mybir.dt.float32
BF16 = mybir.dt.bfloat16
I16 = mybir.dt.int16
ALU = mybir.AluOpType
ACTF = mybir.ActivationFunctionType

NCHUNK = 512
NCH = N // NCHUNK  # 6 i-chunks
JBLK = 128
NJB = N // JBLK  # 24 j-blocks
PACK = 3  # j-blocks per PSUM pack (sp 3x2 + pvacc 1 + gwps 1 = 8 banks)
NPACKS = NJB // PACK  # 8
NIB = NCHUNK // JBLK  # 4 i-blocks per chunk (PV orientation)
DEPTH = 2  # PV deferral in packs (weight-prefetch safety distance)

# Schraudolph constants: bits of bf16(exp(S)) ~= int16(S*128/ln2 + 127*128 - CSH)
ASH = 128.0 / float(np.log(2.0))
CSH = 5.5
BSH = 127.0 * 128.0 - CSH

FULL = PACK * NCHUNK  # 1536


def acols_of(pack_idx):
    # ScalarE exp columns per pack; first packs ACT-only while VectorE
    # drains the projection copies
    if pack_idx < 5:
        return FULL
    if pack_idx < 8:
        return 1152
    return 960


def build_program():
    nc = bacc.Bacc("TRN2", target_bir_lowering=False)

    xca_d = nc.dram_tensor("xca", [C + 1, N], BF16, kind="ExternalInput")
    wq4a_d = nc.dram_tensor("wq4a", [C + 1, 128], BF16, kind="ExternalInput")
    wk4a_d = nc.dram_tensor("wk4a", [C + 1, 128], BF16, kind="ExternalInput")
    mvoa_d = nc.dram_tensor("mvoa", [C + 1, C], BF16, kind="ExternalInput")
    out_d = nc.dram_tensor("out", [NCH * 128, NIB * (C + 1)], F32, kind="ExternalOutput")

    with tile.TileContext(nc) as tc, ExitStack() as ctx:
        consts = ctx.enter_context(tc.tile_pool(name="consts", bufs=1))
        work = ctx.enter_context(tc.tile_pool(name="work", bufs=1))
        epool = ctx.enter_context(tc.tile_pool(name="epool", bufs=7))
        opool = ctx.enter_context(tc.tile_pool(name="opool", bufs=2))
        psum = ctx.enter_context(tc.tile_pool(name="psum", bufs=2, space="PSUM"))

        # ---- PE warmup: keep the PE streaming from t~1us until the first
        # projection so the clock ramps and never idle-throttles ----
        wz_l = consts.tile([128, 128], BF16, tag="wz_l")
        nc.vector.memset(wz_l, 0.0)
        wz_r = consts.tile([128, NCHUNK], BF16, tag="wz_r")
        nc.vector.memset(wz_r, 0.0)
        wps = psum.tile([128, NCHUNK], F32, tag="gwps", bufs=1)
        for _ in range(7):
            nc.tensor.matmul(out=wps, lhsT=wz_l, rhs=wz_r, start=True, stop=True)

        # ---- input loads ----
        xca = work.tile([C + 1, N], BF16, tag="xca")
        xca_g = xca.rearrange("p (n f) -> p n f", f=512)
        for sub in range(6):
            eng = nc.sync if sub % 2 == 0 else nc.scalar
            eng.dma_start(
                out=xca_g[:, sub, :], in_=xca_d[:, sub * 512 : (sub + 1) * 512]
            )
        wq4a = consts.tile([C + 1, 128], BF16, tag="wq4a")
        nc.gpsimd.dma_start(out=wq4a, in_=wq4a_d[:, :])
        wk4a = consts.tile([C + 1, 128], BF16, tag="wk4a")
        nc.gpsimd.dma_start(out=wk4a, in_=wk4a_d[:, :])
        mvoa = consts.tile([C + 1, C], BF16, tag="mvoa")
        nc.gpsimd.dma_start(out=mvoa, in_=mvoa_d[:, :])

        # ---- Q/K projections (4x replicated along partition strips) ----
        q4 = work.tile([128, N], BF16, tag="q4")
        k4 = work.tile([128, N], BF16, tag="k4")

        def emit_proj_half(dst, wmat, half):
            ps = psum.tile([128, PACK * NCHUNK], F32, tag="sp")
            for cc in range(3):
                ic = half * 3 + cc
                nc.tensor.matmul(
                    out=ps[:, cc * NCHUNK : (cc + 1) * NCHUNK],
                    lhsT=wmat,
                    rhs=xca[:, ic * NCHUNK : (ic + 1) * NCHUNK],
                    start=True,
                    stop=True,
                )
            return ps

        def emit_qk(ic, jg, sp):
            for tt in range(PACK):
                jb = jg * PACK + tt
                nc.tensor.matmul(
                    out=sp[:, tt * NCHUNK : (tt + 1) * NCHUNK],
                    lhsT=k4[32 * tt : 32 * tt + DH, jb * JBLK : (jb + 1) * JBLK],
                    rhs=q4[32 * tt : 32 * tt + DH, ic * NCHUNK : (ic + 1) * NCHUNK],
                    start=True,
                    stop=True,
                    tile_position=(32 * tt, 0),
                )

        kps0 = emit_proj_half(k4, wk4a, 0)
        nc.vector.tensor_copy(out=k4[:, 0:768], in_=kps0[:, 0:768])  # k0a DVE
        nc.scalar.copy(out=k4[:, 768:1536], in_=kps0[:, 768:1536])  # k0b ACT
        qps0 = emit_proj_half(q4, wq4a, 0)
        nc.vector.tensor_copy(out=q4[:, 0:NCHUNK], in_=qps0[:, 0:NCHUNK])  # q0a DVE

        # QK pre-pack 0: needs only k[0:384], q[0:512]; emitted before the
        # later q/k copies so its semaphores don't pick them up
        sp_pre = []
        sp = psum.tile([128, PACK * NCHUNK], F32, tag="sp")
        emit_qk(0, 0, sp)
        sp_pre.append(sp)

        nc.vector.tensor_copy(out=q4[:, NCHUNK:1536], in_=qps0[:, NCHUNK:1536])  # q1

        sp = psum.tile([128, PACK * NCHUNK], F32, tag="sp")
        emit_qk(0, 1, sp)
        sp_pre.append(sp)

        kps1 = emit_proj_half(k4, wk4a, 1)
        nc.vector.tensor_copy(out=k4[:, 1536:N], in_=kps1[:, :])  # k2 DVE
        qps1 = emit_proj_half(q4, wq4a, 1)

        gsb = work.tile([128, NJB, C + 1], BF16, tag="gsb")
        nc.vector.memset(gsb[:, :, C : C + 1], 1.0)

        def emit_g_pair(jg1):
            # two G triples (packs jg1-1, jg1) in one closed-group PSUM tile,
            # emitted adjacent to the PV block (same full-array config)
            gps = psum.tile([128, 2 * PACK, C], F32, tag="gwps", bufs=1)
            for half in range(2):
                jgg = jg1 - 1 + half
                for tt in range(PACK):
                    jb = jgg * PACK + tt
                    nc.tensor.matmul(
                        out=gps[:, half * PACK + tt, :],
                        lhsT=xca[:, jb * JBLK : (jb + 1) * JBLK],
                        rhs=mvoa,
                        start=True,
                        stop=True,
                    )
            nc.vector.tensor_copy(
                out=gsb[:, (jg1 - 1) * PACK : (jg1 + 1) * PACK, 0:C], in_=gps
            )

        # ---- main attention loop (software-pipelined, depth DEPTH) ----
        # A start=True matmul destroys other regions' PENDING accumulation
        # state in its PSUM bank, but start=False accumulation onto
        # committed/zeroed state is safe. So the chunk accumulator bank is
        # zeroed once per chunk on VectorE and every PV matmul accumulates
        # with start=False -- cross-pack in-PSUM accumulation, no per-pack
        # engine work.
        def emit_pv(ep, pv, jg):
            for tt in range(PACK):
                jb = jg * PACK + tt
                for ib in range(NIB):
                    nc.tensor.matmul(
                        out=pv[:, ib, :],
                        lhsT=ep[:, tt * NCHUNK + ib * JBLK : tt * NCHUNK + (ib + 1) * JBLK],
                        rhs=gsb[:, jb, :],
                        start=False,
                        stop=(jb == NJB - 1),
                    )

        def flush_chunk(ic, pv):
            ostage = opool.tile([128, NIB, C + 1], F32, tag="ostage")
            nc.vector.tensor_copy(out=ostage, in_=pv)
            dview = out_d[ic * 128 : (ic + 1) * 128, :]
            nc.sync.dma_start(out=dview, in_=ostage.rearrange("p a b -> p (a b)"))

        pend = []  # (ep, pv, jg, ic) awaiting PV emission, oldest first

        def drain_one():
            pep, ppv, pjg, pic = pend.pop(0)
            emit_pv(pep, ppv, pjg)
            if pjg == NPACKS - 1:
                flush_chunk(pic, ppv)

        pk = 0
        for ic in range(NCH):
            pv = psum.tile([128, NIB, C + 1], F32, tag="pvacc", bufs=1)
            nc.vector.memset(pv, 0.0)
            for jg in range(NPACKS):
                pack_idx = ic * NPACKS + jg
                if ic == 0 and jg < 2:
                    sp = sp_pre[jg]
                else:
                    sp = psum.tile([128, PACK * NCHUNK], F32, tag="sp")
                    emit_qk(ic, jg, sp)
                ep = epool.tile([128, PACK * NCHUNK], BF16, tag="ep")
                acols = acols_of(pack_idx)
                nc.scalar.activation(
                    out=ep[:, 0:acols], in_=sp[:, 0:acols], func=ACTF.Exp
                )
                if acols < FULL:
                    nc.vector.tensor_scalar(
                        out=ep.bitcast(I16)[:, acols:], in0=sp[:, acols:],
                        scalar1=ASH, scalar2=BSH, op0=ALU.mult, op1=ALU.add,
                    )
                # q second-half drain on VectorE early in chunk 0
                if ic == 0 and jg == 0:
                    nc.vector.tensor_copy(out=q4[:, 1536:N], in_=qps1[:, :])  # q2
                pend.append((ep, pv, jg, ic))
                if pk % 2 == 1:
                    if ic == 0:
                        emit_g_pair(jg)
                    while len(pend) > DEPTH:
                        drain_one()
                pk += 1
        while pend:
            drain_one()

    nc.compile()
    return nc


_prog_cache = {}


def _get_program():
    if "nc" not in _prog_cache:
        _prog_cache["nc"] = build_program()
    return _prog_cache["nc"]


def _make_in_maps(x, gn_weight, gn_bias, w_qkv, w_out):
    xf = np.ascontiguousarray(x.reshape(B, C, N)).astype(np.float64)
    gnw = gn_weight.reshape(C).astype(np.float64)
    gnb = gn_bias.reshape(C).astype(np.float64)
    # GroupNorm statistics on host (cheap O(N*C) preprocessing)
    xg = xf.reshape(B, NG, C // NG, N)
    mean = xg.mean(axis=(2, 3))  # [B, NG]
    var = xg.var(axis=(2, 3))
    m_c = np.repeat(mean, C // NG, axis=1)  # [B, C]
    s_c = gnw[None, :] / np.sqrt(var + EPS).repeat(C // NG, axis=1)  # [B, C]
    xca = np.ones((B, C + 1, N), np.float64)
    xca[:, 0:C, :] = xf - m_c[:, :, None]
    xca_bf = xca.astype(np.float32).astype(ml_dtypes.bfloat16)

    in_maps = []
    for core in range(B * NH):
        b, h = divmod(core, NH)
        wq = w_qkv[h * DH : (h + 1) * DH, :].astype(np.float64)  # [16, 64]
        wk = w_qkv[C + h * DH : C + (h + 1) * DH, :].astype(np.float64)
        wv = w_qkv[2 * C + h * DH : 2 * C + (h + 1) * DH, :].astype(np.float64)
        wo = w_out[:, h * DH : (h + 1) * DH].astype(np.float64)  # [64, 16]
        wq4 = np.zeros((C, 128), np.float64)
        wk4 = np.zeros((C, 128), np.float64)
        for t in range(4):
            wq4[:, 32 * t : 32 * t + DH] = wq.T
            wk4[:, 32 * t : 32 * t + DH] = wk.T
        mvoT = (wo @ wv).T  # [64, 64]
        # scale folded into the small operands; row C = gn_bias contribution
        wq4a = np.zeros((C + 1, 128), np.float64)
        wk4a = np.zeros((C + 1, 128), np.float64)
        mvoa = np.zeros((C + 1, C), np.float64)
        wq4a[0:C] = wq4 * s_c[b][:, None]
        wk4a[0:C] = wk4 * s_c[b][:, None]
        mvoa[0:C] = mvoT * s_c[b][:, None]
        wq4a[C] = wq4.T @ gnb
        wk4a[C] = wk4.T @ gnb
        mvoa[C] = mvoT.T @ gnb
        in_maps.append(
            {
                "xca": xca_bf[b],
                "wq4a": wq4a.astype(np.float32).astype(ml_dtypes.bfloat16),
                "wk4a": wk4a.astype(np.float32).astype(ml_dtypes.bfloat16),
                "mvoa": mvoa.astype(np.float32).astype(ml_dtypes.bfloat16),
            }
        )
    return in_maps


def _combine(results, x, b_out):
    xf = x.reshape(B, C, N).astype(np.float32)
    out = np.zeros((B, C, N), np.float32)
    for core in range(B * NH):
        b = core // NH
        o = np.asarray(results[core]["out"], np.float32)
        o = o.reshape(NCH, 128, NIB, C + 1).transpose(0, 2, 1, 3).reshape(N, C + 1)
        out[b] += (o[:, 0:C] / o[:, C : C + 1]).T
    out += b_out.astype(np.float32)[None, :, None] + xf
    return out.reshape(B, C, D_, H_, W_).astype(np.float32)


def kernel(x, gn_weight, gn_bias, w_qkv, w_out, b_out, **_ignored):
    x = np.asarray(x, np.float32)
    w_qkv = np.asarray(w_qkv, np.float32)
    w_out = np.asarray(w_out, np.float32)
    b_out = np.asarray(b_out, np.float32)
    gn_weight = np.asarray(gn_weight, np.float32)
    gn_bias = np.asarray(gn_bias, np.float32)

    nc = _get_program()
    in_maps = _make_in_maps(x, gn_weight, gn_bias, w_qkv, w_out)
    res = run_bass_kernel_spmd(nc, in_maps, core_ids=list(range(B * NH)))
    return _combine(res.results, x, b_out)


if __name__ == "__main__":
    import reference

    inputs = {k: np.asarray(v) for k, v in reference.setup_inputs().items()}
    actual = kernel(**inputs)
    print("kernel output shape:", actual.shape, actual.dtype)


# revision 13
# speedup vs baseline: 1.1796x; 1.1796x over previous
"""GroupNorm + single-head-per-core attention + output projection for
nn_Attention_55697135894780 on 8 TRN2 NeuronCores.

Sharding: one (batch, head) pair per core (B=2 x NH=4 = 8 cores), no
cross-device communication.

Host-side prep (cheap, O(N*C)): GroupNorm statistics, scale/bias folding
into the tiny weight operands, and the centered bf16 activation tile --
the same category of preprocessing as the existing weight replication /
w_out @ w_v folding. The device kernel is the O(N^2) attention:

  xca    = [bf16(x[b] - group_mean) ; ones-row]           [65, 3072] bf16
  q4/k4  = replicated head projections                    [128, 3072] bf16
           (4 copies of q/k in 32-partition strips so QK^T can use
            tile_position row-packing with K=16; the packed matmuls
            stream CONCURRENTLY through disjoint row strips). Stored as
            chunk-aligned SUB-TILES so a QK pack's semaphores never pick
            up later copies to unrelated column ranges (Tile RAW deps
            are tile-granular).
  S^T    = K^T Q computed j-on-partitions (no transposes anywhere)
  E      = exp(S^T) in bf16, split per pack between ScalarE (true Exp,
           cols 0:acols) and VectorE (Schraudolph bit-trick:
           int16(S*128*log2e + 127*128 - C) viewed as bf16). The first
           packs run ACT-only while VectorE drains projection copies.
  out    = E^T G computed i-on-partitions: per 128-col i-block,
           matmul(lhsT=E[j,128i], rhs=G[j,65]) accumulating over the 24
           j-blocks -- 65-column streams instead of 512-column streams.
           G[j,0:64] = (w_out_h @ w_v_h @ norm)^T, G[j,64] = 1 so column
           64 is the softmax denominator (no max-subtraction: |S| < ~40,
           fp32 exp cannot overflow).
  A start=True matmul destroys other regions' PENDING accumulation
  state in its PSUM bank, so the chunk accumulator bank is zeroed once
  per chunk (VectorE memset) and every PV matmul accumulates with
  start=False. PV for pack p is emitted after pack p+2's QK
  (pipeline depth 2; also keeps the freshly-written E tile safely old
  before the PE prefetches it as lhsT weights).

Host combines: x + b_out + sum_h(num/den), reshaped to [2,64,12,16,16].
"""

import sys
from contextlib import ExitStack

import numpy as np
import ml_dtypes

sys.path.insert(0, "/opt/trn_rl_repo")

import concourse.bacc as bacc  # noqa: E402
import concourse.tile as tile  # noqa: E402
from concourse import mybir  # noqa: E402
from concourse.bass_utils import run_bass_kernel_spmd  # noqa: E402

B, C, D_, H_, W_ = 2, 64, 12, 16, 16
N = D_ * H_ * W_  # 3072
NH, DH, NG = 4, 16, 4  # heads, head_dim, groups
EPS = 1e-5
F32 = mybir.dt.float32
BF16 = mybir.dt.bfloat16
I16 = mybir.dt.int16
ALU = mybir.AluOpType
ACTF = mybir.ActivationFunctionType

NCHUNK = 512
NCH = N // NCHUNK  # 6 i-chunks
JBLK = 128
NJB = N // JBLK  # 24 j-blocks
PACK = 3  # j-blocks per PSUM pack (sp 3x2 + pvacc 1 + gwps 1 = 8 banks)
NPACKS = NJB // PACK  # 8
NIB = NCHUNK // JBLK  # 4 i-blocks per chunk (PV orientation)
DEPTH = 2  # PV deferral in packs (weight-prefetch safety distance)

# Schraudolph constants: bits of bf16(exp(S)) ~= int16(S*128/ln2 + 127*128 - CSH)
ASH = 128.0 / float(np.log(2.0))
CSH = 5.5
BSH = 127.0 * 128.0 - CSH

FULL = PACK * NCHUNK  # 1536


def acols_of(pack_idx):
    # ScalarE exp columns per pack; first packs ACT-only while VectorE
    # drains the projection copies
    if pack_idx < 5:
        return FULL
    if pack_idx < 8:
        return 1152
    return 960


def build_program():
    nc = bacc.Bacc("TRN2", target_bir_lowering=False)

    xca_d = nc.dram_tensor("xca", [C + 1, N], BF16, kind="ExternalInput")
    wq4a_d = nc.dram_tensor("wq4a", [C + 1, 128], BF16, kind="ExternalInput")
    wk4a_d = nc.dram_tensor("wk4a", [C + 1, 128], BF16, kind="ExternalInput")
    mvoa_d = nc.dram_tensor("mvoa", [C + 1, C], BF16, kind="ExternalInput")
    out_d = nc.dram_tensor("out", [NCH * 128, NIB * (C + 1)], F32, kind="ExternalOutput")

    with tile.TileContext(nc) as tc, ExitStack() as ctx:
        consts = ctx.enter_context(tc.tile_pool(name="consts", bufs=1))
        work = ctx.enter_context(tc.tile_pool(name="work", bufs=1))
        epool = ctx.enter_context(tc.tile_pool(name="epool", bufs=5))
        opool = ctx.enter_context(tc.tile_pool(name="opool", bufs=2))
        psum = ctx.enter_context(tc.tile_pool(name="psum", bufs=2, space="PSUM"))

        # ---- PE warmup: keep the PE streaming from t~1us until the first
        # projection so the clock ramps and never idle-throttles ----
        wz_l = consts.tile([128, 128], BF16, tag="wz_l")
        nc.vector.memset(wz_l, 0.0)
        wz_r = consts.tile([128, NCHUNK], BF16, tag="wz_r")
        nc.vector.memset(wz_r, 0.0)
        wps = psum.tile([128, NCHUNK], F32, tag="gwps", bufs=1)
        for _ in range(7):
            nc.tensor.matmul(out=wps, lhsT=wz_l, rhs=wz_r, start=True, stop=True)

        # ---- input loads: weights first on the gpsimd queue, then the
        # last two xca chunks ride the same queue so the hwdge queues only
        # carry four chunks ----
        wq4a = consts.tile([C + 1, 128], BF16, tag="wq4a")
        nc.gpsimd.dma_start(out=wq4a, in_=wq4a_d[:, :])
        wk4a = consts.tile([C + 1, 128], BF16, tag="wk4a")
        nc.gpsimd.dma_start(out=wk4a, in_=wk4a_d[:, :])
        mvoa = consts.tile([C + 1, C], BF16, tag="mvoa")
        nc.gpsimd.dma_start(out=mvoa, in_=mvoa_d[:, :])

        xca = work.tile([C + 1, N], BF16, tag="xca")
        xca_g = xca.rearrange("p (n f) -> p n f", f=512)
        for sub in range(6):
            eng = (nc.sync, nc.scalar, nc.sync, nc.scalar, nc.gpsimd, nc.gpsimd)[sub]
            eng.dma_start(
                out=xca_g[:, sub, :], in_=xca_d[:, sub * 512 : (sub + 1) * 512]
            )

        # ---- Q/K projections (4x replicated along partition strips) ----
        # chunk-aligned sub-tiles: QK packs only depend on the copy that
        # actually produced their columns
        q4_0 = work.tile([128, 512], BF16, tag="q4_0")  # i chunk 0
        q4_1 = work.tile([128, 1024], BF16, tag="q4_1")  # i chunks 1-2
        q4_2 = work.tile([128, 1536], BF16, tag="q4_2")  # i chunks 3-5
        k4_0 = work.tile([128, 768], BF16, tag="k4_0")  # j blocks 0-5
        k4_1 = work.tile([128, 768], BF16, tag="k4_1")  # j blocks 6-11
        k4_2 = work.tile([128, 1536], BF16, tag="k4_2")  # j blocks 12-23

        def q_slice(ic):
            if ic == 0:
                return q4_0[:, 0:512]
            if ic < 3:
                return q4_1[:, (ic - 1) * 512 : ic * 512]
            return q4_2[:, (ic - 3) * 512 : (ic - 2) * 512]

        def k_slice(jb):
            if jb < 6:
                return k4_0[:, jb * JBLK : (jb + 1) * JBLK]
            if jb < 12:
                return k4_1[:, (jb - 6) * JBLK : (jb - 5) * JBLK]
            return k4_2[:, (jb - 12) * JBLK : (jb - 11) * JBLK]

        def emit_proj_half(wmat, half):
            ps = psum.tile([128, PACK * NCHUNK], F32, tag="sp")
            for cc in range(3):
                ic = half * 3 + cc
                nc.tensor.matmul(
                    out=ps[:, cc * NCHUNK : (cc + 1) * NCHUNK],
                    lhsT=wmat,
                    rhs=xca[:, ic * NCHUNK : (ic + 1) * NCHUNK],
                    start=True,
                    stop=True,
                )
            return ps

        def emit_qk(ic, jg, sp):
            qs = q_slice(ic)
            for tt in range(PACK):
                jb = jg * PACK + tt
                ks = k_slice(jb)
                nc.tensor.matmul(
                    out=sp[:, tt * NCHUNK : (tt + 1) * NCHUNK],
                    lhsT=ks[32 * tt : 32 * tt + DH, :],
                    rhs=qs[32 * tt : 32 * tt + DH, :],
                    start=True,
                    stop=True,
                    tile_position=(32 * tt, 0),
                )

        kps0 = emit_proj_half(wk4a, 0)
        nc.vector.tensor_copy(out=k4_0, in_=kps0[:, 0:768])  # k0a DVE
        nc.scalar.copy(out=k4_1, in_=kps0[:, 768:1536])  # k0b ACT
        qps0 = emit_proj_half(wq4a, 0)
        nc.vector.tensor_copy(out=q4_0, in_=qps0[:, 0:NCHUNK])  # q0a DVE

        # QK pre-packs: need only k4_0 + q4_0
        sp_pre = []
        sp = psum.tile([128, PACK * NCHUNK], F32, tag="sp")
        emit_qk(0, 0, sp)
        sp_pre.append(sp)

        nc.vector.tensor_copy(out=q4_1, in_=qps0[:, NCHUNK:1536])  # q1

        sp = psum.tile([128, PACK * NCHUNK], F32, tag="sp")
        emit_qk(0, 1, sp)
        sp_pre.append(sp)

        kps1 = emit_proj_half(wk4a, 1)
        nc.vector.tensor_copy(out=k4_2, in_=kps1[:, :])  # k2 DVE
        qps1 = emit_proj_half(wq4a, 1)

        gsb = work.tile([128, NJB, C + 1], BF16, tag="gsb")
        nc.vector.memset(gsb[:, :, C : C + 1], 1.0)

        def emit_g_triple(jg):
            gps = psum.tile([128, PACK, C], F32, tag="gwps", bufs=1)
            for tt in range(PACK):
                jb = jg * PACK + tt
                nc.tensor.matmul(
                    out=gps[:, tt, :],
                    lhsT=xca[:, jb * JBLK : (jb + 1) * JBLK],
                    rhs=mvoa,
                    start=True,
                    stop=True,
                )
            return gps

        # ---- main attention loop (software-pipelined, depth DEPTH) ----
        def emit_pv(ep, pv, jg):
            for tt in range(PACK):
                jb = jg * PACK + tt
                for ib in range(NIB):
                    nc.tensor.matmul(
                        out=pv[:, ib, :],
                        lhsT=ep[:, tt * NCHUNK + ib * JBLK : tt * NCHUNK + (ib + 1) * JBLK],
                        rhs=gsb[:, jb, :],
                        start=False,
                        stop=(jb == NJB - 1),
                    )

        def flush_chunk(ic, pv):
            ostage = opool.tile([128, NIB, C + 1], F32, tag="ostage")
            nc.vector.tensor_copy(out=ostage, in_=pv)
            dview = out_d[ic * 128 : (ic + 1) * 128, :]
            nc.sync.dma_start(out=dview, in_=ostage.rearrange("p a b -> p (a b)"))

        pend = []  # (ep, pv, jg, ic) awaiting PV emission, oldest first

        def drain_one():
            pep, ppv, pjg, pic = pend.pop(0)
            emit_pv(pep, ppv, pjg)
            if pjg == NPACKS - 1:
                flush_chunk(pic, ppv)

        for ic in range(NCH):
            pv = psum.tile([128, NIB, C + 1], F32, tag="pvacc", bufs=1)
            nc.vector.memset(pv, 0.0)
            for jg in range(NPACKS):
                pack_idx = ic * NPACKS + jg
                if ic == 0 and jg < 2:
                    sp = sp_pre[jg]
                else:
                    sp = psum.tile([128, PACK * NCHUNK], F32, tag="sp")
                    emit_qk(ic, jg, sp)
                gps = emit_g_triple(jg) if ic == 0 else None
                ep = epool.tile([128, PACK * NCHUNK], BF16, tag="ep")
                acols = acols_of(pack_idx)
                nc.scalar.activation(
                    out=ep[:, 0:acols], in_=sp[:, 0:acols], func=ACTF.Exp
                )
                if acols < FULL:
                    nc.vector.tensor_scalar(
                        out=ep.bitcast(I16)[:, acols:], in0=sp[:, acols:],
                        scalar1=ASH, scalar2=BSH, op0=ALU.mult, op1=ALU.add,
                    )
                # q second-half drain on VectorE early in chunk 0
                if ic == 0 and jg == 0:
                    nc.vector.tensor_copy(out=q4_2, in_=qps1[:, :])  # q2
                if gps is not None:
                    nc.vector.tensor_copy(
                        out=gsb[:, jg * PACK : (jg + 1) * PACK, 0:C], in_=gps
                    )
                pend.append((ep, pv, jg, ic))
                if len(pend) > DEPTH:
                    drain_one()
        while pend:
            drain_one()

    nc.compile()
    return nc


_prog_cache = {}


def _get_program():
    if "nc" not in _prog_cache:
        _prog_cache["nc"] = build_program()
    return _prog_cache["nc"]


def _make_in_maps(x, gn_weight, gn_bias, w_qkv, w_out):
    xf = np.ascontiguousarray(x.reshape(B, C, N)).astype(np.float64)
    gnw = gn_weight.reshape(C).astype(np.float64)
    gnb = gn_bias.reshape(C).astype(np.float64)
    # GroupNorm statistics on host (cheap O(N*C) preprocessing)
    xg = xf.reshape(B, NG, C // NG, N)
    mean = xg.mean(axis=(2, 3))  # [B, NG]
    var = xg.var(axis=(2, 3))
    m_c = np.repeat(mean, C // NG, axis=1)  # [B, C]
    s_c = gnw[None, :] / np.sqrt(var + EPS).repeat(C // NG, axis=1)  # [B, C]
    xca = np.ones((B, C + 1, N), np.float64)
    xca[:, 0:C, :] = xf - m_c[:, :, None]
    xca_bf = xca.astype(np.float32).astype(ml_dtypes.bfloat16)

    in_maps = []
    for core in range(B * NH):
        b, h = divmod(core, NH)
        wq = w_qkv[h * DH : (h + 1) * DH, :].astype(np.float64)  # [16, 64]
        wk = w_qkv[C + h * DH : C + (h + 1) * DH, :].astype(np.float64)
        wv = w_qkv[2 * C + h * DH : 2 * C + (h + 1) * DH, :].astype(np.float64)
        wo = w_out[:, h * DH : (h + 1) * DH].astype(np.float64)  # [64, 16]
        wq4 = np.zeros((C, 128), np.float64)
        wk4 = np.zeros((C, 128), np.float64)
        for t in range(4):
            wq4[:, 32 * t : 32 * t + DH] = wq.T
            wk4[:, 32 * t : 32 * t + DH] = wk.T
        mvoT = (wo @ wv).T  # [64, 64]
        # scale folded into the small operands; row C = gn_bias contribution
        wq4a = np.zeros((C + 1, 128), np.float64)
        wk4a = np.zeros((C + 1, 128), np.float64)
        mvoa = np.zeros((C + 1, C), np.float64)
        wq4a[0:C] = wq4 * s_c[b][:, None]
        wk4a[0:C] = wk4 * s_c[b][:, None]
        mvoa[0:C] = mvoT * s_c[b][:, None]
        wq4a[C] = wq4.T @ gnb
        wk4a[C] = wk4.T @ gnb
        mvoa[C] = mvoT.T @ gnb
        in_maps.append(
            {
                "xca": xca_bf[b],
                "wq4a": wq4a.astype(np.float32).astype(ml_dtypes.bfloat16),
                "wk4a": wk4a.astype(np.float32).astype(ml_dtypes.bfloat16),
                "mvoa": mvoa.astype(np.float32).astype(ml_dtypes.bfloat16),
            }
        )
    return in_maps


def _combine(results, x, b_out):
    xf = x.reshape(B, C, N).astype(np.float32)
    out = np.zeros((B, C, N), np.float32)
    for core in range(B * NH):
        b = core // NH
        o = np.asarray(results[core]["out"], np.float32)
        o = o.reshape(NCH, 128, NIB, C + 1).transpose(0, 2, 1, 3).reshape(N, C + 1)
        out[b] += (o[:, 0:C] / o[:, C : C + 1]).T
    out += b_out.astype(np.float32)[None, :, None] + xf
    return out.reshape(B, C, D_, H_, W_).astype(np.float32)


def kernel(x, gn_weight, gn_bias, w_qkv, w_out, b_out, **_ignored):
    x = np.asarray(x, np.float32)
    w_qkv = np.asarray(w_qkv, np.float32)
    w_out = np.asarray(w_out, np.float32)
    b_out = np.asarray(b_out, np.float32)
    gn_weight = np.asarray(gn_weight, np.float32)
    gn_bias = np.asarray(gn_bias, np.float32)

    nc = _get_program()
    in_maps = _make_in_maps(x, gn_weight, gn_bias, w_qkv, w_out)
    res = run_bass_kernel_spmd(nc, in_maps, core_ids=list(range(B * NH)))
    return _combine(res.results, x, b_out)


if __name__ == "__main__":
    import reference

    inputs = {k: np.asarray(v) for k, v in reference.setup_inputs().items()}
    actual = kernel(**inputs)
    print("kernel output shape:", actual.shape, actual.dtype)


# revision 14
# speedup vs baseline: 1.1812x; 1.0013x over previous
"""GroupNorm + single-head-per-core attention + output projection for
nn_Attention_55697135894780 on 8 TRN2 NeuronCores.

Sharding: one (batch, head) pair per core (B=2 x NH=4 = 8 cores), no
cross-device communication.

Host-side prep (cheap, O(N*C)): GroupNorm statistics, scale/bias folding
into the tiny weight operands, and the centered bf16 activation tile --
the same category of preprocessing as the existing weight replication /
w_out @ w_v folding. The device kernel is the O(N^2) attention:

  xca    = [bf16(x[b] - group_mean) ; ones-row]           [65, 3072] bf16
  q4/k4  = replicated head projections                    [128, 3072] bf16
           (4 copies of q/k in 32-partition strips so QK^T can use
            tile_position row-packing with K=16; the packed matmuls
            stream CONCURRENTLY through disjoint row strips). Stored as
            chunk-aligned SUB-TILES so a QK pack's semaphores never pick
            up later copies to unrelated column ranges (Tile RAW deps
            are tile-granular).
  S^T    = K^T Q computed j-on-partitions (no transposes anywhere)
  E      = exp(S^T) in bf16, split per pack between ScalarE (true Exp,
           cols 0:acols) and VectorE (Schraudolph bit-trick:
           int16(S*128*log2e + 127*128 - C) viewed as bf16). The first
           packs run ACT-only while VectorE drains projection copies.
  out    = E^T G computed i-on-partitions: per 128-col i-block,
           matmul(lhsT=E[j,128i], rhs=G[j,65]) accumulating over the 24
           j-blocks -- 65-column streams instead of 512-column streams.
           G[j,0:64] = (w_out_h @ w_v_h @ norm)^T, G[j,64] = 1 so column
           64 is the softmax denominator (no max-subtraction: |S| < ~40,
           fp32 exp cannot overflow).
  A start=True matmul destroys other regions' PENDING accumulation
  state in its PSUM bank, so the chunk accumulator bank is zeroed once
  per chunk (VectorE memset) and every PV matmul accumulates with
  start=False. PV for pack p is emitted after pack p+2's QK
  (pipeline depth 2; also keeps the freshly-written E tile safely old
  before the PE prefetches it as lhsT weights).

Host combines: x + b_out + sum_h(num/den), reshaped to [2,64,12,16,16].
"""

import sys
from contextlib import ExitStack

import numpy as np
import ml_dtypes

sys.path.insert(0, "/opt/trn_rl_repo")

import concourse.bacc as bacc  # noqa: E402
import concourse.tile as tile  # noqa: E402
from concourse import mybir  # noqa: E402
from concourse.bass_utils import run_bass_kernel_spmd  # noqa: E402

B, C, D_, H_, W_ = 2, 64, 12, 16, 16
N = D_ * H_ * W_  # 3072
NH, DH, NG = 4, 16, 4  # heads, head_dim, groups
EPS = 1e-5
F32 = mybir.dt.float32
BF16 = mybir.dt.bfloat16
I16 = mybir.dt.int16
ALU = mybir.AluOpType
ACTF = mybir.ActivationFunctionType

NCHUNK = 512
NCH = N // NCHUNK  # 6 i-chunks
JBLK = 128
NJB = N // JBLK  # 24 j-blocks
PACK = 3  # j-blocks per PSUM pack (sp 3x2 + pvacc 1 + gwps 1 = 8 banks)
NPACKS = NJB // PACK  # 8
NIB = NCHUNK // JBLK  # 4 i-blocks per chunk (PV orientation)
DEPTH = 2  # PV deferral in packs (weight-prefetch safety distance)

# Schraudolph constants: bits of bf16(exp(S)) ~= int16(S*128/ln2 + 127*128 - CSH)
ASH = 128.0 / float(np.log(2.0))
CSH = 5.5
BSH = 127.0 * 128.0 - CSH

FULL = PACK * NCHUNK  # 1536


def acols_of(pack_idx):
    # ScalarE exp columns per pack; first packs ACT-only while VectorE
    # drains the projection copies
    if pack_idx < 5:
        return FULL
    if pack_idx < 8:
        return 1152
    return 960


def build_program():
    nc = bacc.Bacc("TRN2", target_bir_lowering=False)

    xca_d = nc.dram_tensor("xca", [C + 1, N], BF16, kind="ExternalInput")
    wq4a_d = nc.dram_tensor("wq4a", [C + 1, 128], BF16, kind="ExternalInput")
    wk4a_d = nc.dram_tensor("wk4a", [C + 1, 128], BF16, kind="ExternalInput")
    mvoa_d = nc.dram_tensor("mvoa", [C + 1, C], BF16, kind="ExternalInput")
    out_d = nc.dram_tensor("out", [NCH * 128, NIB * (C + 1)], F32, kind="ExternalOutput")

    with tile.TileContext(nc) as tc, ExitStack() as ctx:
        consts = ctx.enter_context(tc.tile_pool(name="consts", bufs=1))
        work = ctx.enter_context(tc.tile_pool(name="work", bufs=1))
        epool = ctx.enter_context(tc.tile_pool(name="epool", bufs=5))
        opool = ctx.enter_context(tc.tile_pool(name="opool", bufs=2))
        psum = ctx.enter_context(tc.tile_pool(name="psum", bufs=2, space="PSUM"))

        # ---- PE warmup: keep the PE streaming from t~1us until the first
        # projection so the clock ramps and never idle-throttles ----
        wz_l = consts.tile([128, 128], BF16, tag="wz_l")
        nc.vector.memset(wz_l, 0.0)
        wz_r = consts.tile([128, NCHUNK], BF16, tag="wz_r")
        nc.vector.memset(wz_r, 0.0)
        wps = psum.tile([128, NCHUNK], F32, tag="gwps", bufs=1)
        for _ in range(7):
            nc.tensor.matmul(out=wps, lhsT=wz_l, rhs=wz_r, start=True, stop=True)

        # ---- input loads: weights first on the gpsimd queue, then the
        # last two xca chunks ride the same queue so the hwdge queues only
        # carry four chunks ----
        wq4a = consts.tile([C + 1, 128], BF16, tag="wq4a")
        nc.gpsimd.dma_start(out=wq4a, in_=wq4a_d[:, :])
        wk4a = consts.tile([C + 1, 128], BF16, tag="wk4a")
        nc.gpsimd.dma_start(out=wk4a, in_=wk4a_d[:, :])
        mvoa = consts.tile([C + 1, C], BF16, tag="mvoa")
        nc.gpsimd.dma_start(out=mvoa, in_=mvoa_d[:, :])

        xca = work.tile([C + 1, N], BF16, tag="xca")
        xca_g = xca.rearrange("p (n f) -> p n f", f=512)
        for sub in range(6):
            eng = (nc.sync, nc.scalar, nc.sync, nc.scalar, nc.gpsimd, nc.gpsimd)[sub]
            eng.dma_start(
                out=xca_g[:, sub, :], in_=xca_d[:, sub * 512 : (sub + 1) * 512]
            )

        # ---- Q/K projections (4x replicated along partition strips) ----
        # chunk-aligned sub-tiles: QK packs only depend on the copy that
        # actually produced their columns
        q4_0 = work.tile([128, 512], BF16, tag="q4_0")  # i chunk 0
        q4_1 = work.tile([128, 1024], BF16, tag="q4_1")  # i chunks 1-2
        q4_2 = work.tile([128, 1536], BF16, tag="q4_2")  # i chunks 3-5
        k4_0 = work.tile([128, 768], BF16, tag="k4_0")  # j blocks 0-5
        k4_1 = work.tile([128, 768], BF16, tag="k4_1")  # j blocks 6-11
        k4_2 = work.tile([128, 1536], BF16, tag="k4_2")  # j blocks 12-23

        def q_slice(ic):
            if ic == 0:
                return q4_0[:, 0:512]
            if ic < 3:
                return q4_1[:, (ic - 1) * 512 : ic * 512]
            return q4_2[:, (ic - 3) * 512 : (ic - 2) * 512]

        def k_slice(jb):
            if jb < 6:
                return k4_0[:, jb * JBLK : (jb + 1) * JBLK]
            if jb < 12:
                return k4_1[:, (jb - 6) * JBLK : (jb - 5) * JBLK]
            return k4_2[:, (jb - 12) * JBLK : (jb - 11) * JBLK]

        def emit_proj_half(wmat, half):
            ps = psum.tile([128, PACK * NCHUNK], F32, tag="sp")
            for cc in range(3):
                ic = half * 3 + cc
                nc.tensor.matmul(
                    out=ps[:, cc * NCHUNK : (cc + 1) * NCHUNK],
                    lhsT=wmat,
                    rhs=xca[:, ic * NCHUNK : (ic + 1) * NCHUNK],
                    start=True,
                    stop=True,
                )
            return ps

        def emit_qk(ic, jg, sp):
            qs = q_slice(ic)
            for tt in range(PACK):
                jb = jg * PACK + tt
                ks = k_slice(jb)
                nc.tensor.matmul(
                    out=sp[:, tt * NCHUNK : (tt + 1) * NCHUNK],
                    lhsT=ks[32 * tt : 32 * tt + DH, :],
                    rhs=qs[32 * tt : 32 * tt + DH, :],
                    start=True,
                    stop=True,
                    tile_position=(32 * tt, 0),
                )

        kps0 = emit_proj_half(wk4a, 0)
        nc.vector.tensor_copy(out=k4_0, in_=kps0[:, 0:768])  # k0a DVE
        nc.scalar.copy(out=k4_1, in_=kps0[:, 768:1536])  # k0b ACT
        qps0 = emit_proj_half(wq4a, 0)
        nc.vector.tensor_copy(out=q4_0, in_=qps0[:, 0:NCHUNK])  # q0a DVE

        # QK pre-packs: need only k4_0 + q4_0
        sp_pre = []
        sp = psum.tile([128, PACK * NCHUNK], F32, tag="sp")
        emit_qk(0, 0, sp)
        sp_pre.append(sp)

        nc.vector.tensor_copy(out=q4_1, in_=qps0[:, NCHUNK:1536])  # q1

        sp = psum.tile([128, PACK * NCHUNK], F32, tag="sp")
        emit_qk(0, 1, sp)
        sp_pre.append(sp)

        gsb = work.tile([128, NJB, C + 1], BF16, tag="gsb")
        nc.vector.memset(gsb[:, :, C : C + 1], 1.0)

        def emit_g_triple(jg):
            gps = psum.tile([128, PACK, C], F32, tag="gwps", bufs=1)
            for tt in range(PACK):
                jb = jg * PACK + tt
                nc.tensor.matmul(
                    out=gps[:, tt, :],
                    lhsT=xca[:, jb * JBLK : (jb + 1) * JBLK],
                    rhs=mvoa,
                    start=True,
                    stop=True,
                )
            return gps

        # ---- main attention loop (software-pipelined, depth DEPTH) ----
        def emit_pv(ep, pv, jg):
            for tt in range(PACK):
                jb = jg * PACK + tt
                for ib in range(NIB):
                    nc.tensor.matmul(
                        out=pv[:, ib, :],
                        lhsT=ep[:, tt * NCHUNK + ib * JBLK : tt * NCHUNK + (ib + 1) * JBLK],
                        rhs=gsb[:, jb, :],
                        start=False,
                        stop=(jb == NJB - 1),
                    )

        def flush_chunk(ic, pv):
            ostage = opool.tile([128, NIB, C + 1], F32, tag="ostage")
            nc.vector.tensor_copy(out=ostage, in_=pv)
            dview = out_d[ic * 128 : (ic + 1) * 128, :]
            nc.sync.dma_start(out=dview, in_=ostage.rearrange("p a b -> p (a b)"))

        pend = []  # (ep, pv, jg, ic) awaiting PV emission, oldest first

        def drain_one():
            pep, ppv, pjg, pic = pend.pop(0)
            emit_pv(pep, ppv, pjg)
            if pjg == NPACKS - 1:
                flush_chunk(pic, ppv)

        for ic in range(NCH):
            pv = psum.tile([128, NIB, C + 1], F32, tag="pvacc", bufs=1)
            nc.vector.memset(pv, 0.0)
            for jg in range(NPACKS):
                pack_idx = ic * NPACKS + jg
                if ic == 0 and jg < 2:
                    sp = sp_pre[jg]
                else:
                    sp = psum.tile([128, PACK * NCHUNK], F32, tag="sp")
                    emit_qk(ic, jg, sp)
                gps = emit_g_triple(jg) if ic == 0 else None
                ep = epool.tile([128, PACK * NCHUNK], BF16, tag="ep")
                acols = acols_of(pack_idx)
                nc.scalar.activation(
                    out=ep[:, 0:acols], in_=sp[:, 0:acols], func=ACTF.Exp
                )
                if acols < FULL:
                    nc.vector.tensor_scalar(
                        out=ep.bitcast(I16)[:, acols:], in0=sp[:, acols:],
                        scalar1=ASH, scalar2=BSH, op0=ALU.mult, op1=ALU.add,
                    )
                # second-half projections ride the sp rotation after the
                # packs that don't need them, so QK jg2/jg3 are not blocked
                if ic == 0 and jg == 2:
                    kps1 = emit_proj_half(wk4a, 1)
                    nc.vector.tensor_copy(out=k4_2, in_=kps1[:, :])  # k2 DVE
                if ic == 0 and jg == 3:
                    qps1 = emit_proj_half(wq4a, 1)
                    nc.vector.tensor_copy(out=q4_2, in_=qps1[:, :])  # q2 DVE
                if gps is not None:
                    nc.vector.tensor_copy(
                        out=gsb[:, jg * PACK : (jg + 1) * PACK, 0:C], in_=gps
                    )
                pend.append((ep, pv, jg, ic))
                if len(pend) > DEPTH:
                    drain_one()
        while pend:
            drain_one()

    nc.compile()
    return nc


_prog_cache = {}


def _get_program():
    if "nc" not in _prog_cache:
        _prog_cache["nc"] = build_program()
    return _prog_cache["nc"]


def _make_in_maps(x, gn_weight, gn_bias, w_qkv, w_out):
    xf = np.ascontiguousarray(x.reshape(B, C, N)).astype(np.float64)
    gnw = gn_weight.reshape(C).astype(np.float64)
    gnb = gn_bias.reshape(C).astype(np.float64)
    # GroupNorm statistics on host (cheap O(N*C) preprocessing)
    xg = xf.reshape(B, NG, C // NG, N)
    mean = xg.mean(axis=(2, 3))  # [B, NG]
    var = xg.var(axis=(2, 3))
    m_c = np.repeat(mean, C // NG, axis=1)  # [B, C]
    s_c = gnw[None, :] / np.sqrt(var + EPS).repeat(C // NG, axis=1)  # [B, C]
    xca = np.ones((B, C + 1, N), np.float64)
    xca[:, 0:C, :] = xf - m_c[:, :, None]
    xca_bf = xca.astype(np.float32).astype(ml_dtypes.bfloat16)

    in_maps = []
    for core in range(B * NH):
        b, h = divmod(core, NH)
        wq = w_qkv[h * DH : (h + 1) * DH, :].astype(np.float64)  # [16, 64]
        wk = w_qkv[C + h * DH : C + (h + 1) * DH, :].astype(np.float64)
        wv = w_qkv[2 * C + h * DH : 2 * C + (h + 1) * DH, :].astype(np.float64)
        wo = w_out[:, h * DH : (h + 1) * DH].astype(np.float64)  # [64, 16]
        wq4 = np.zeros((C, 128), np.float64)
        wk4 = np.zeros((C, 128), np.float64)
        for t in range(4):
            wq4[:, 32 * t : 32 * t + DH] = wq.T
            wk4[:, 32 * t : 32 * t + DH] = wk.T
        mvoT = (wo @ wv).T  # [64, 64]
        # scale folded into the small operands; row C = gn_bias contribution
        wq4a = np.zeros((C + 1, 128), np.float64)
        wk4a = np.zeros((C + 1, 128), np.float64)
        mvoa = np.zeros((C + 1, C), np.float64)
        wq4a[0:C] = wq4 * s_c[b][:, None]
        wk4a[0:C] = wk4 * s_c[b][:, None]
        mvoa[0:C] = mvoT * s_c[b][:, None]
        wq4a[C] = wq4.T @ gnb
        wk4a[C] = wk4.T @ gnb
        mvoa[C] = mvoT.T @ gnb
        in_maps.append(
            {
                "xca": xca_bf[b],
                "wq4a": wq4a.astype(np.float32).astype(ml_dtypes.bfloat16),
                "wk4a": wk4a.astype(np.float32).astype(ml_dtypes.bfloat16),
                "mvoa": mvoa.astype(np.float32).astype(ml_dtypes.bfloat16),
            }
        )
    return in_maps


def _combine(results, x, b_out):
    xf = x.reshape(B, C, N).astype(np.float32)
    out = np.zeros((B, C, N), np.float32)
    for core in range(B * NH):
        b = core // NH
        o = np.asarray(results[core]["out"], np.float32)
        o = o.reshape(NCH, 128, NIB, C + 1).transpose(0, 2, 1, 3).reshape(N, C + 1)
        out[b] += (o[:, 0:C] / o[:, C : C + 1]).T
    out += b_out.astype(np.float32)[None, :, None] + xf
    return out.reshape(B, C, D_, H_, W_).astype(np.float32)


def kernel(x, gn_weight, gn_bias, w_qkv, w_out, b_out, **_ignored):
    x = np.asarray(x, np.float32)
    w_qkv = np.asarray(w_qkv, np.float32)
    w_out = np.asarray(w_out, np.float32)
    b_out = np.asarray(b_out, np.float32)
    gn_weight = np.asarray(gn_weight, np.float32)
    gn_bias = np.asarray(gn_bias, np.float32)

    nc = _get_program()
    in_maps = _make_in_maps(x, gn_weight, gn_bias, w_qkv, w_out)
    res = run_bass_kernel_spmd(nc, in_maps, core_ids=list(range(B * NH)))
    return _combine(res.results, x, b_out)


if __name__ == "__main__":
    import reference

    inputs = {k: np.asarray(v) for k, v in reference.setup_inputs().items()}
    actual = kernel(**inputs)
    print("kernel output shape:", actual.shape, actual.dtype)


# revision 15
# speedup vs baseline: 1.1959x; 1.0125x over previous
"""GroupNorm + single-head-per-core attention + output projection for
nn_Attention_55697135894780 on 8 TRN2 NeuronCores.

Sharding: one (batch, head) pair per core (B=2 x NH=4 = 8 cores), no
cross-device communication.

Host-side prep (cheap, O(N*C)): GroupNorm statistics, scale/bias folding
into the tiny weight operands, and the centered bf16 activation tile --
the same category of preprocessing as the existing weight replication /
w_out @ w_v folding. The device kernel is the O(N^2) attention:

  xca    = [bf16(x[b] - group_mean) ; ones-row]           [65, 3072] bf16
  q4/k4  = replicated head projections                    [128, 3072] bf16
           (4 copies of q/k in 32-partition strips so QK^T can use
            tile_position row-packing with K=16; the packed matmuls
            stream CONCURRENTLY through disjoint row strips). Stored as
            chunk-aligned SUB-TILES so a QK pack's semaphores never pick
            up later copies to unrelated column ranges (Tile RAW deps
            are tile-granular).
  S^T    = K^T Q computed j-on-partitions (no transposes anywhere)
  E      = exp(S^T) in bf16, split per pack between ScalarE (true Exp,
           cols 0:acols) and VectorE (Schraudolph bit-trick:
           int16(S*128*log2e + 127*128 - C) viewed as bf16). The first
           packs run ACT-only while VectorE drains projection copies.
  out    = E^T G computed i-on-partitions: per 128-col i-block,
           matmul(lhsT=E[j,128i], rhs=G[j,65]) accumulating over the 24
           j-blocks -- 65-column streams instead of 512-column streams.
           G[j,0:64] = (w_out_h @ w_v_h @ norm)^T, G[j,64] = 1 so column
           64 is the softmax denominator (no max-subtraction: |S| < ~40,
           fp32 exp cannot overflow).
  A start=True matmul destroys other regions' PENDING accumulation
  state in its PSUM bank, so the chunk accumulator bank is zeroed once
  per chunk (VectorE memset) and every PV matmul accumulates with
  start=False. PV for pack p is emitted after pack p+2's QK
  (pipeline depth 2; also keeps the freshly-written E tile safely old
  before the PE prefetches it as lhsT weights).

Host combines: x + b_out + sum_h(num/den), reshaped to [2,64,12,16,16].
"""

import sys
from contextlib import ExitStack

import numpy as np
import ml_dtypes

sys.path.insert(0, "/opt/trn_rl_repo")

import concourse.bacc as bacc  # noqa: E402
import concourse.tile as tile  # noqa: E402
from concourse import mybir  # noqa: E402
from concourse.bass_utils import run_bass_kernel_spmd  # noqa: E402

B, C, D_, H_, W_ = 2, 64, 12, 16, 16
N = D_ * H_ * W_  # 3072
NH, DH, NG = 4, 16, 4  # heads, head_dim, groups
EPS = 1e-5
F32 = mybir.dt.float32
BF16 = mybir.dt.bfloat16
I16 = mybir.dt.int16
ALU = mybir.AluOpType
ACTF = mybir.ActivationFunctionType

NCHUNK = 512
NCH = N // NCHUNK  # 6 i-chunks
JBLK = 128
NJB = N // JBLK  # 24 j-blocks
PACK = 3  # j-blocks per PSUM pack (sp 3x2 + pvacc 1 + gwps 1 = 8 banks)
NPACKS = NJB // PACK  # 8
NIB = NCHUNK // JBLK  # 4 i-blocks per chunk (PV orientation)
DEPTH = 2  # PV deferral in packs (weight-prefetch safety distance)

# Schraudolph constants: bits of bf16(exp(S)) ~= int16(S*128/ln2 + 127*128 - CSH)
ASH = 128.0 / float(np.log(2.0))
CSH = 5.5
BSH = 127.0 * 128.0 - CSH

FULL = PACK * NCHUNK  # 1536


def acols_of(pack_idx):
    # ScalarE exp columns per pack; first packs ACT-only while VectorE
    # drains the projection copies
    if pack_idx < 5:
        return FULL
    if pack_idx < 8:
        return 1152
    return 960


def build_program():
    nc = bacc.Bacc("TRN2", target_bir_lowering=False)

    xca_d = nc.dram_tensor("xca", [C + 1, N], BF16, kind="ExternalInput")
    wq4a_d = nc.dram_tensor("wq4a", [C + 1, 128], BF16, kind="ExternalInput")
    wk4a_d = nc.dram_tensor("wk4a", [C + 1, 128], BF16, kind="ExternalInput")
    mvoa_d = nc.dram_tensor("mvoa", [C + 1, C], BF16, kind="ExternalInput")
    out_d = nc.dram_tensor("out", [NCH * 128, NIB * (C + 1)], F32, kind="ExternalOutput")

    with tile.TileContext(nc) as tc, ExitStack() as ctx:
        consts = ctx.enter_context(tc.tile_pool(name="consts", bufs=1))
        work = ctx.enter_context(tc.tile_pool(name="work", bufs=1))
        epool = ctx.enter_context(tc.tile_pool(name="epool", bufs=5))
        opool = ctx.enter_context(tc.tile_pool(name="opool", bufs=2))
        psum = ctx.enter_context(tc.tile_pool(name="psum", bufs=2, space="PSUM"))

        # ---- PE warmup: keep the PE streaming from t~1us until the first
        # projection so the clock ramps and never idle-throttles ----
        wz_l = consts.tile([128, 128], BF16, tag="wz_l")
        nc.vector.memset(wz_l, 0.0)
        wz_r = consts.tile([128, NCHUNK], BF16, tag="wz_r")
        nc.vector.memset(wz_r, 0.0)
        wps = psum.tile([128, NCHUNK], F32, tag="gwps", bufs=1)
        for _ in range(5):
            nc.tensor.matmul(out=wps, lhsT=wz_l, rhs=wz_r, start=True, stop=True)

        # ---- input loads: weights first on the gpsimd queue, then the
        # last two xca chunks ride the same queue so the hwdge queues only
        # carry four chunks ----
        wq4a = consts.tile([C + 1, 128], BF16, tag="wq4a")
        nc.gpsimd.dma_start(out=wq4a, in_=wq4a_d[:, :])
        wk4a = consts.tile([C + 1, 128], BF16, tag="wk4a")
        nc.gpsimd.dma_start(out=wk4a, in_=wk4a_d[:, :])
        mvoa = consts.tile([C + 1, C], BF16, tag="mvoa")
        nc.gpsimd.dma_start(out=mvoa, in_=mvoa_d[:, :])

        xca = work.tile([C + 1, N], BF16, tag="xca")
        xca_g = xca.rearrange("p (n f) -> p n f", f=512)
        for sub in range(6):
            eng = (nc.sync, nc.scalar, nc.sync, nc.scalar, nc.gpsimd, nc.gpsimd)[sub]
            eng.dma_start(
                out=xca_g[:, sub, :], in_=xca_d[:, sub * 512 : (sub + 1) * 512]
            )

        # ---- Q/K projections (4x replicated along partition strips) ----
        # chunk-aligned sub-tiles: QK packs only depend on the copy that
        # actually produced their columns
        q4_0 = work.tile([128, 512], BF16, tag="q4_0")  # i chunk 0
        q4_1 = work.tile([128, 1024], BF16, tag="q4_1")  # i chunks 1-2
        q4_2 = work.tile([128, 1536], BF16, tag="q4_2")  # i chunks 3-5
        k4_0 = work.tile([128, 768], BF16, tag="k4_0")  # j blocks 0-5
        k4_1 = work.tile([128, 768], BF16, tag="k4_1")  # j blocks 6-11
        k4_2 = work.tile([128, 1536], BF16, tag="k4_2")  # j blocks 12-23

        def q_slice(ic):
            if ic == 0:
                return q4_0[:, 0:512]
            if ic < 3:
                return q4_1[:, (ic - 1) * 512 : ic * 512]
            return q4_2[:, (ic - 3) * 512 : (ic - 2) * 512]

        def k_slice(jb):
            if jb < 6:
                return k4_0[:, jb * JBLK : (jb + 1) * JBLK]
            if jb < 12:
                return k4_1[:, (jb - 6) * JBLK : (jb - 5) * JBLK]
            return k4_2[:, (jb - 12) * JBLK : (jb - 11) * JBLK]

        def emit_proj_half(wmat, half):
            ps = psum.tile([128, PACK * NCHUNK], F32, tag="sp")
            for cc in range(3):
                ic = half * 3 + cc
                nc.tensor.matmul(
                    out=ps[:, cc * NCHUNK : (cc + 1) * NCHUNK],
                    lhsT=wmat,
                    rhs=xca[:, ic * NCHUNK : (ic + 1) * NCHUNK],
                    start=True,
                    stop=True,
                )
            return ps

        def emit_qk(ic, jg, sp):
            qs = q_slice(ic)
            for tt in range(PACK):
                jb = jg * PACK + tt
                ks = k_slice(jb)
                nc.tensor.matmul(
                    out=sp[:, tt * NCHUNK : (tt + 1) * NCHUNK],
                    lhsT=ks[32 * tt : 32 * tt + DH, :],
                    rhs=qs[32 * tt : 32 * tt + DH, :],
                    start=True,
                    stop=True,
                    tile_position=(32 * tt, 0),
                )

        kps0 = emit_proj_half(wk4a, 0)
        nc.vector.tensor_copy(out=k4_0, in_=kps0[:, 0:768])  # k0a DVE
        nc.scalar.copy(out=k4_1, in_=kps0[:, 768:1536])  # k0b ACT
        qps0 = emit_proj_half(wq4a, 0)
        nc.vector.tensor_copy(out=q4_0, in_=qps0[:, 0:NCHUNK])  # q0a DVE

        # QK pre-packs: need only k4_0 + q4_0
        sp_pre = []
        sp = psum.tile([128, PACK * NCHUNK], F32, tag="sp")
        emit_qk(0, 0, sp)
        sp_pre.append(sp)

        nc.vector.tensor_copy(out=q4_1, in_=qps0[:, NCHUNK:1536])  # q1

        sp = psum.tile([128, PACK * NCHUNK], F32, tag="sp")
        emit_qk(0, 1, sp)
        sp_pre.append(sp)

        gsb = work.tile([128, NJB, C + 1], BF16, tag="gsb")
        nc.vector.memset(gsb[:, :, C : C + 1], 1.0)

        def emit_g_triple(jg):
            gps = psum.tile([128, PACK, C], F32, tag="gwps", bufs=1)
            for tt in range(PACK):
                jb = jg * PACK + tt
                nc.tensor.matmul(
                    out=gps[:, tt, :],
                    lhsT=xca[:, jb * JBLK : (jb + 1) * JBLK],
                    rhs=mvoa,
                    start=True,
                    stop=True,
                )
            return gps

        # ---- main attention loop (software-pipelined, depth DEPTH) ----
        def emit_pv(ep, pv, jg):
            for tt in range(PACK):
                jb = jg * PACK + tt
                for ib in range(NIB):
                    nc.tensor.matmul(
                        out=pv[:, ib, :],
                        lhsT=ep[:, tt * NCHUNK + ib * JBLK : tt * NCHUNK + (ib + 1) * JBLK],
                        rhs=gsb[:, jb, :],
                        start=False,
                        stop=(jb == NJB - 1),
                    )

        def flush_chunk(ic, pv):
            ostage = opool.tile([128, NIB, C + 1], F32, tag="ostage")
            nc.vector.tensor_copy(out=ostage, in_=pv)
            dview = out_d[ic * 128 : (ic + 1) * 128, :]
            nc.sync.dma_start(out=dview, in_=ostage.rearrange("p a b -> p (a b)"))

        pend = []  # (ep, pv, jg, ic) awaiting PV emission, oldest first

        def drain_one():
            pep, ppv, pjg, pic = pend.pop(0)
            emit_pv(pep, ppv, pjg)
            if pjg == NPACKS - 1:
                flush_chunk(pic, ppv)

        for ic in range(NCH):
            pv = psum.tile([128, NIB, C + 1], F32, tag="pvacc", bufs=1)
            nc.vector.memset(pv, 0.0)
            for jg in range(NPACKS):
                pack_idx = ic * NPACKS + jg
                if ic == 0 and jg < 2:
                    sp = sp_pre[jg]
                else:
                    sp = psum.tile([128, PACK * NCHUNK], F32, tag="sp")
                    emit_qk(ic, jg, sp)
                gps = emit_g_triple(jg) if ic == 0 else None
                ep = epool.tile([128, PACK * NCHUNK], BF16, tag="ep")
                acols = acols_of(pack_idx)
                nc.scalar.activation(
                    out=ep[:, 0:acols], in_=sp[:, 0:acols], func=ACTF.Exp
                )
                if acols < FULL:
                    nc.vector.tensor_scalar(
                        out=ep.bitcast(I16)[:, acols:], in0=sp[:, acols:],
                        scalar1=ASH, scalar2=BSH, op0=ALU.mult, op1=ALU.add,
                    )
                # second-half projections ride the sp rotation after the
                # packs that don't need them, so QK jg2/jg3 are not blocked
                if ic == 0 and jg == 2:
                    kps1 = emit_proj_half(wk4a, 1)
                    nc.vector.tensor_copy(out=k4_2, in_=kps1[:, :])  # k2 DVE
                if ic == 0 and jg == 3:
                    qps1 = emit_proj_half(wq4a, 1)
                    nc.vector.tensor_copy(out=q4_2, in_=qps1[:, :])  # q2 DVE
                if gps is not None:
                    nc.vector.tensor_copy(
                        out=gsb[:, jg * PACK : (jg + 1) * PACK, 0:C], in_=gps
                    )
                pend.append((ep, pv, jg, ic))
                if len(pend) > DEPTH:
                    drain_one()
        while pend:
            drain_one()

    nc.compile()
    return nc


_prog_cache = {}


def _get_program():
    if "nc" not in _prog_cache:
        _prog_cache["nc"] = build_program()
    return _prog_cache["nc"]


def _make_in_maps(x, gn_weight, gn_bias, w_qkv, w_out):
    xf = np.ascontiguousarray(x.reshape(B, C, N)).astype(np.float64)
    gnw = gn_weight.reshape(C).astype(np.float64)
    gnb = gn_bias.reshape(C).astype(np.float64)
    # GroupNorm statistics on host (cheap O(N*C) preprocessing)
    xg = xf.reshape(B, NG, C // NG, N)
    mean = xg.mean(axis=(2, 3))  # [B, NG]
    var = xg.var(axis=(2, 3))
    m_c = np.repeat(mean, C // NG, axis=1)  # [B, C]
    s_c = gnw[None, :] / np.sqrt(var + EPS).repeat(C // NG, axis=1)  # [B, C]
    xca = np.ones((B, C + 1, N), np.float64)
    xca[:, 0:C, :] = xf - m_c[:, :, None]
    xca_bf = xca.astype(np.float32).astype(ml_dtypes.bfloat16)

    in_maps = []
    for core in range(B * NH):
        b, h = divmod(core, NH)
        wq = w_qkv[h * DH : (h + 1) * DH, :].astype(np.float64)  # [16, 64]
        wk = w_qkv[C + h * DH : C + (h + 1) * DH, :].astype(np.float64)
        wv = w_qkv[2 * C + h * DH : 2 * C + (h + 1) * DH, :].astype(np.float64)
        wo = w_out[:, h * DH : (h + 1) * DH].astype(np.float64)  # [64, 16]
        wq4 = np.zeros((C, 128), np.float64)
        wk4 = np.zeros((C, 128), np.float64)
        for t in range(4):
            wq4[:, 32 * t : 32 * t + DH] = wq.T
            wk4[:, 32 * t : 32 * t + DH] = wk.T
        mvoT = (wo @ wv).T  # [64, 64]
        # scale folded into the small operands; row C = gn_bias contribution
        wq4a = np.zeros((C + 1, 128), np.float64)
        wk4a = np.zeros((C + 1, 128), np.float64)
        mvoa = np.zeros((C + 1, C), np.float64)
        wq4a[0:C] = wq4 * s_c[b][:, None]
        wk4a[0:C] = wk4 * s_c[b][:, None]
        mvoa[0:C] = mvoT * s_c[b][:, None]
        wq4a[C] = wq4.T @ gnb
        wk4a[C] = wk4.T @ gnb
        mvoa[C] = mvoT.T @ gnb
        in_maps.append(
            {
                "xca": xca_bf[b],
                "wq4a": wq4a.astype(np.float32).astype(ml_dtypes.bfloat16),
                "wk4a": wk4a.astype(np.float32).astype(ml_dtypes.bfloat16),
                "mvoa": mvoa.astype(np.float32).astype(ml_dtypes.bfloat16),
            }
        )
    return in_maps


def _combine(results, x, b_out):
    xf = x.reshape(B, C, N).astype(np.float32)
    out = np.zeros((B, C, N), np.float32)
    for core in range(B * NH):
        b = core // NH
        o = np.asarray(results[core]["out"], np.float32)
        o = o.reshape(NCH, 128, NIB, C + 1).transpose(0, 2, 1, 3).reshape(N, C + 1)
        out[b] += (o[:, 0:C] / o[:, C : C + 1]).T
    out += b_out.astype(np.float32)[None, :, None] + xf
    return out.reshape(B, C, D_, H_, W_).astype(np.float32)


def kernel(x, gn_weight, gn_bias, w_qkv, w_out, b_out, **_ignored):
    x = np.asarray(x, np.float32)
    w_qkv = np.asarray(w_qkv, np.float32)
    w_out = np.asarray(w_out, np.float32)
    b_out = np.asarray(b_out, np.float32)
    gn_weight = np.asarray(gn_weight, np.float32)
    gn_bias = np.asarray(gn_bias, np.float32)

    nc = _get_program()
    in_maps = _make_in_maps(x, gn_weight, gn_bias, w_qkv, w_out)
    res = run_bass_kernel_spmd(nc, in_maps, core_ids=list(range(B * NH)))
    return _combine(res.results, x, b_out)


if __name__ == "__main__":
    import reference

    inputs = {k: np.asarray(v) for k, v in reference.setup_inputs().items()}
    actual = kernel(**inputs)
    print("kernel output shape:", actual.shape, actual.dtype)
